# revision 6
# baseline (speedup 1.0000x reference)
"""Trainium2 Bass kernel for nn_ControlFlowExpert_62380105007397.

Reference semantics (CPU-XLA eager jax):
  x: [16, 8192, 208] fp32.
  imm = sequential fp32 chain sum_n x[..., 195+n] * 16^n   (n = 0..7)
  pc  = same over cols 171..178
  ax  = int32-wrap sum of trunc-toward-zero casts of cols 163..170 times 16^n
  any_jmp/any_bz/any_bnz = global any() of opcode cols 90/92/93 > 0.5
  If any flag set: out = x with cols 171..178 = nibbles of int32(new_pc)
  and col 203 = branch-taken flag; else out = x.

Strategy: the op reads only 8 input columns and modifies only 9 output
columns; everything else is identity. Flags are computed on host (3
column scans) and select a compile-time specialized device kernel. The
dominant any_jmp path: host slices the 8 imm columns into a compact
[rows, 8] fp32 array per core (batch-sharded over 8 cores); the device
computes the exact sequential fp32 chain (bit-identical to XLA CPU),
truncates toward zero (RNE cast + sign-bit-or correction, HW-verified
exact), and extracts all 8 nibbles with 4 uint16-bitcast shift-and ops,
writing compact nibble planes. Host assembles out = x.copy() with the
device nibbles spliced into cols 171..178 and col 203 = 1. Device HBM
traffic drops 37x vs streaming all 208 columns. Rare paths (bz/bnz
without jmp) use a host-computed 9-column patch spliced on device.
"""

import sys

if "/opt/trn_rl_repo" not in sys.path:
    sys.path.insert(0, "/opt/trn_rl_repo")

import numpy as np

B, T, C = 16, 8192, 208
N_CORES = 8
ROWS_PER_CORE = (B // N_CORES) * T          # 16384
P = 128                                     # SBUF partitions
W = ROWS_PER_CORE // P                      # 128 rows per partition

OPC_JMP, OPC_BZ, OPC_BNZ = 90, 92, 93
AX0, PC0, IMM0, BT = 163, 171, 195, 203

_kernel_cache = {}

# perf knobs (test harness overrides these before first kernel() call)
CONFIG = {"tiles": 2, "bufs": 2, "out_engine": "scalar", "impl": "raw"}


def _emit_compact(nc, mybir, pool, sp, a3, nibt, tag):
    """DVE pipeline: a3 [P, Wt, 8] fp32 view -> nibt [P, 4, 2*Wt] u16 view.

    chain (7 STT, exact sequential fp32 order) -> trunc toward zero
    (9 ops, HW-verified) -> 4 shift-and nibble ops on the u16 bitcast.
    """
    A = mybir.AluOpType
    f32, i32, u32, u16 = (mybir.dt.float32, mybir.dt.int32, mybir.dt.uint32,
                          mybir.dt.uint16)
    ws = a3.shape[1]

    acc = sp.tile([P, ws], f32, tag=f"acc{tag}")
    nc.vector.scalar_tensor_tensor(
        out=acc[:], in0=a3[:, :, 1], scalar=16.0,
        in1=a3[:, :, 0], op0=A.mult, op1=A.add)
    for n in range(2, 8):
        nacc = sp.tile([P, ws], f32, tag=f"acc{n}{tag}")
        nc.vector.scalar_tensor_tensor(
            out=nacc[:], in0=a3[:, :, n], scalar=float(16.0 ** n),
            in1=acc[:], op0=A.mult, op1=A.add)
        acc = nacc

    # trunc toward zero: y = rne(acc); fy = f(y); d = acc - fy; w = d*fy;
    # corr = or(bits(w<0), signbit(fy)); ft = fy - corr; v = i32(ft)
    y = sp.tile([P, ws], i32, tag=f"y{tag}")
    nc.vector.tensor_copy(out=y[:], in_=acc[:])
    fy = sp.tile([P, ws], f32, tag=f"fy{tag}")
    nc.vector.tensor_copy(out=fy[:], in_=y[:])
    d = sp.tile([P, ws], f32, tag=f"d{tag}")
    nc.vector.scalar_tensor_tensor(
        out=d[:], in0=fy[:], scalar=-1.0, in1=acc[:], op0=A.mult, op1=A.add)
    w = sp.tile([P, ws], f32, tag=f"w{tag}")
    nc.vector.tensor_tensor(out=w[:], in0=d[:], in1=fy[:], op=A.mult)
    mask = sp.tile([P, ws], f32, tag=f"mask{tag}")
    nc.vector.tensor_scalar(out=mask[:], in0=w[:], scalar1=0.0, scalar2=None,
                            op0=A.is_lt)
    sb = sp.tile([P, ws], u32, tag=f"sb{tag}")
    nc.vector.tensor_scalar(out=sb[:], in0=fy[:].bitcast(u32),
                            scalar1=0x80000000, scalar2=None,
                            op0=A.bitwise_and)
    corr = sp.tile([P, ws], u32, tag=f"corr{tag}")
    nc.vector.tensor_tensor(out=corr[:], in0=mask[:].bitcast(u32),
                            in1=sb[:], op=A.bitwise_or)
    ft = sp.tile([P, ws], f32, tag=f"ft{tag}")
    nc.vector.tensor_tensor(out=ft[:], in0=fy[:], in1=corr[:].bitcast(f32),
                            op=A.subtract)
    v = sp.tile([P, ws], i32, tag=f"v{tag}")
    nc.vector.tensor_copy(out=v[:], in_=ft[:])

    # nibbles: u16 view [P, 2*ws]: elem (2w+j) = halfword j of row w.
    # plane n holds nibble n (j=0) and nibble n+4 (j=1) of each row.
    v16 = v[:].bitcast(u16)
    nc.vector.tensor_scalar(out=nibt[:, 0, :], in0=v16, scalar1=15,
                            scalar2=None, op0=A.bitwise_and)
    for n in range(1, 4):
        nc.vector.tensor_scalar(out=nibt[:, n, :], in0=v16,
                                scalar1=4 * n, scalar2=15,
                                op0=A.logical_shift_right,
                                op1=A.bitwise_and)


def _build_jmp_compact():
    """any_jmp path: compact columns in, nibble planes out."""
    import concourse.bacc as bacc
    import concourse.mybir as mybir
    from concourse.tile import TileContext

    f32 = mybir.dt.float32
    u16 = mybir.dt.uint16
    nt = CONFIG["tiles"]
    wt = W // nt                    # rows per partition per tile
    tile_rows = P * wt

    nc = bacc.Bacc("TRN2")
    out_eng = getattr(nc, CONFIG["out_engine"])
    a = nc.dram_tensor("a", [ROWS_PER_CORE, 8], f32, kind="ExternalInput")
    onib = nc.dram_tensor("onib", [nt, P, 8 * wt], u16, kind="ExternalOutput")

    with TileContext(nc) as tc:
        with tc.tile_pool(name="sbuf", bufs=CONFIG["bufs"]) as pool, \
             tc.tile_pool(name="small", bufs=2) as sp:
            for t in range(nt):
                rows = slice(t * tile_rows, (t + 1) * tile_rows)
                at = pool.tile([P, wt * 8], f32, tag="at")
                nc.sync.dma_start(
                    out=at[:],
                    in_=a[rows, :].rearrange("(p w) c -> p (w c)", p=P))
                a3 = at[:].rearrange("p (w c) -> p w c", c=8)
                nibt = pool.tile([P, 8 * wt], u16, tag="nibt")
                n3 = nibt[:].rearrange("p (n w) -> p n w", n=4)
                _emit_compact(nc, mybir, pool, sp, a3, n3, tag="")
                out_eng.dma_start(out=onib[t], in_=nibt[:])
    nc.finalize()
    return nc


def _build_jmp_raw():
    """any_jmp path, raw bass: no TileContext epilogue, manual sems,
    two interleaved row-streams so RAW-dependent DVE ops never wait on
    pipeline drains (the other stream's op provides the spacing)."""
    from contextlib import ExitStack

    import concourse.bacc as bacc
    import concourse.mybir as mybir

    A = mybir.AluOpType
    f32, i32, u32, u16 = (mybir.dt.float32, mybir.dt.int32, mybir.dt.uint32,
                          mybir.dt.uint16)
    NS = CONFIG["tiles"]            # streams
    ws = W // NS                    # rows per partition per stream
    srows = P * ws

    nc = bacc.Bacc("TRN2")
    a = nc.dram_tensor("a", [ROWS_PER_CORE, 8], f32, kind="ExternalInput")
    onib = nc.dram_tensor("onib", [NS, P, 8 * ws], u16, kind="ExternalOutput")

    with ExitStack() as st:
        ats = [st.enter_context(nc.sbuf_tensor(f"at{s}", [P, ws * 8], f32))
               for s in range(NS)]
        nibs = [st.enter_context(nc.sbuf_tensor(f"nib{s}", [P, 8 * ws], u16))
                for s in range(NS)]
        tmp = []
        for s in range(NS):
            t = {}
            for k in ("accA", "accB", "fy", "d", "w", "mask", "ft"):
                t[k] = st.enter_context(
                    nc.sbuf_tensor(f"t{s}_{k}", [P, ws], f32))
            for k in ("y", "v"):
                t[k] = st.enter_context(
                    nc.sbuf_tensor(f"t{s}_{k}", [P, ws], i32))
            for k in ("sb", "corr"):
                t[k] = st.enter_context(
                    nc.sbuf_tensor(f"t{s}_{k}", [P, ws], u32))
            tmp.append(t)
        sem_in = st.enter_context(nc.semaphore("sin"))
        sem_cmp = st.enter_context(nc.semaphore("scmp"))
        sem_out = st.enter_context(nc.semaphore("sout"))
        block = st.enter_context(nc.Block())

        @block.sync
        def _(sync):
            for s in range(NS):
                rows = slice(s * srows, (s + 1) * srows)
                sync.dma_start(
                    ats[s][:],
                    a[rows, :].rearrange("(p w) c -> p (w c)", p=P),
                ).then_inc(sem_in, 16)

        def steps(s):
            """Yield the 20-op DVE program for stream s as thunks."""
            t = tmp[s]
            a3 = ats[s][:].rearrange("p (w c) -> p w c", c=8)
            acc_cur, acc_nxt = t["accA"], t["accB"]

            def op_chain0(acc_cur=acc_cur):
                return nc.vector.scalar_tensor_tensor(
                    out=acc_cur[:], in0=a3[:, :, 1], scalar=16.0,
                    in1=a3[:, :, 0], op0=A.mult, op1=A.add)
            yield op_chain0
            for n in range(2, 8):
                def op_chainN(n=n, dst=acc_nxt, src=acc_cur):
                    return nc.vector.scalar_tensor_tensor(
                        out=dst[:], in0=a3[:, :, n], scalar=float(16.0 ** n),
                        in1=src[:], op0=A.mult, op1=A.add)
                yield op_chainN
                acc_cur, acc_nxt = acc_nxt, acc_cur
            acc = acc_cur
            yield lambda: nc.vector.tensor_copy(out=t["y"][:], in_=acc[:])
            yield lambda: nc.vector.tensor_copy(out=t["fy"][:], in_=t["y"][:])
            yield lambda: nc.vector.scalar_tensor_tensor(
                out=t["d"][:], in0=t["fy"][:], scalar=-1.0, in1=acc[:],
                op0=A.mult, op1=A.add)
            yield lambda: nc.vector.tensor_tensor(
                out=t["w"][:], in0=t["d"][:], in1=t["fy"][:], op=A.mult)
            yield lambda: nc.vector.tensor_scalar(
                out=t["mask"][:], in0=t["w"][:], scalar1=0.0, scalar2=None,
                op0=A.is_lt)
            yield lambda: nc.vector.tensor_scalar(
                out=t["sb"][:], in0=t["fy"][:].bitcast(u32),
                scalar1=0x80000000, scalar2=None, op0=A.bitwise_and)
            yield lambda: nc.vector.tensor_tensor(
                out=t["corr"][:], in0=t["mask"][:].bitcast(u32),
                in1=t["sb"][:], op=A.bitwise_or)
            yield lambda: nc.vector.tensor_tensor(
                out=t["ft"][:], in0=t["fy"][:], in1=t["corr"][:].bitcast(f32),
                op=A.subtract)
            yield lambda: nc.vector.tensor_copy(out=t["v"][:], in_=t["ft"][:])
            v16 = t["v"][:].bitcast(u16)
            n3 = nibs[s][:].rearrange("p (n w) -> p n w", n=4)

            def op_nib0():
                return nc.vector.tensor_scalar(
                    out=n3[:, 0, :], in0=v16, scalar1=15, scalar2=None,
                    op0=A.bitwise_and)
            yield op_nib0
            for n in range(1, 4):
                def op_nibN(n=n):
                    return nc.vector.tensor_scalar(
                        out=n3[:, n, :], in0=v16, scalar1=4 * n, scalar2=15,
                        op0=A.logical_shift_right, op1=A.bitwise_and)
                yield op_nibN

        @block.vector
        def _(vector):
            vector.wait_ge(sem_in, 16 * NS)
            gens = [steps(s) for s in range(NS)]
            done = [False] * NS
            lasts = [None] * NS
            while not all(done):
                for s in range(NS):
                    if done[s]:
                        continue
                    try:
                        lasts[s] = next(gens[s])()
                    except StopIteration:
                        done[s] = True
                        # drain flushes the DVE pipe so the out-DMA sees
                        # committed SBUF data before sem_cmp fires
                        nc.vector.drain().then_inc(sem_cmp, 1)

        @block.scalar
        def _(scalar):
            for s in range(NS):
                scalar.wait_ge(sem_cmp, s + 1)
                scalar.dma_start(onib[s], nibs[s][:]).then_inc(sem_out, 16)
            scalar.wait_ge(sem_out, 16 * NS)

    nc.finalize()
    return nc


def _build_patch_kernel():
    """Device kernel for rare flag combos: stream x, splice host patch."""
    import concourse.bacc as bacc
    import concourse.mybir as mybir
    from concourse.tile import TileContext

    f32 = mybir.dt.float32
    W16 = 16
    TILE_ROWS = P * W16
    N_TILES = ROWS_PER_CORE // TILE_ROWS

    nc = bacc.Bacc("TRN2")
    x = nc.dram_tensor("x", [ROWS_PER_CORE, C], f32, kind="ExternalInput")
    patch = nc.dram_tensor("patch", [ROWS_PER_CORE, 9], f32, kind="ExternalInput")
    out = nc.dram_tensor("out", [ROWS_PER_CORE, C], f32, kind="ExternalOutput")

    with TileContext(nc) as tc:
        with tc.tile_pool(name="sbuf", bufs=4) as pool, \
             tc.tile_pool(name="small", bufs=3) as sp:
            for t in range(N_TILES):
                rows = slice(t * TILE_ROWS, (t + 1) * TILE_ROWS)
                xt = pool.tile([P, W16 * C], f32, tag="xt")
                x3 = xt[:].rearrange("p (w c) -> p w c", c=C)
                nc.sync.dma_start(
                    out=xt[:],
                    in_=x[rows, :].rearrange("(p w) c -> p (w c)", p=P))
                pt = sp.tile([P, W16 * 9], f32, tag="pt")
                p3 = pt[:].rearrange("p (w c) -> p w c", c=9)
                nc.sync.dma_start(
                    out=pt[:],
                    in_=patch[rows, :].rearrange("(p w) c -> p (w c)", p=P))
                nc.vector.tensor_copy(out=x3[:, :, PC0:PC0 + 8], in_=p3[:, :, 0:8])
                nc.vector.tensor_copy(out=x3[:, :, BT], in_=p3[:, :, 8])
                nc.sync.dma_start(
                    out=out[rows, :].rearrange("(p w) c -> p (w c)", p=P),
                    in_=xt[:])
    nc.finalize()
    return nc


def _get_kernel(name):
    if name not in _kernel_cache:
        if name == "jmp":
            builder = (_build_jmp_raw if CONFIG.get("impl") == "raw"
                       else _build_jmp_compact)
            _kernel_cache[name] = builder()
        else:
            _kernel_cache[name] = _build_patch_kernel()
    return _kernel_cache[name]


# test.py can set _RUN_KWARGS["trace"] = True and read LAST for profiling.
_RUN_KWARGS = {}
LAST = None


def _run_spmd(nc, in_maps):
    global LAST
    from concourse.bass_utils import run_bass_kernel_spmd
    LAST = run_bass_kernel_spmd(nc, in_maps, core_ids=list(range(N_CORES)),
                                **_RUN_KWARGS)
    return LAST


def _host_patch(x):
    """Exact CPU-XLA-equivalent computation of the 9 modified columns."""
    pw = np.float32(16.0) ** np.arange(8, dtype=np.float32)
    imm = x[..., IMM0].astype(np.float32)
    pc = x[..., PC0].astype(np.float32)
    for n in range(1, 8):
        imm = (x[..., IMM0 + n] * pw[n] + imm).astype(np.float32)
        pc = (x[..., PC0 + n] * pw[n] + pc).astype(np.float32)
    axs = np.zeros(x.shape[:-1], dtype=np.int64)
    for n in range(8):
        axs += x[..., AX0 + n].astype(np.int32).astype(np.int64) * (16 ** n)
    ax = ((axs + 2**31) % 2**32 - 2**31).astype(np.int32)
    ax_is_zero = ax == 0

    any_bz = bool((x[..., OPC_BZ] > 0.5).any())

    pc8 = (pc + np.float32(8.0)).astype(np.float32)
    if any_bz:
        new_pc = np.where(ax_is_zero, imm, pc8)
        bt = ax_is_zero.astype(np.float32)
    else:  # any_bnz
        new_pc = np.where(~ax_is_zero, imm, pc8)
        bt = (~ax_is_zero).astype(np.float32)
    v = new_pc.astype(np.int32)
    shifts = np.arange(8, dtype=np.int32) * 4
    nibs = ((v[..., None] >> shifts) & 15).astype(np.float32)
    return np.concatenate([nibs, bt[..., None]], axis=-1)


def kernel(x):
    x = np.ascontiguousarray(np.asarray(x), dtype=np.float32)
    assert x.shape == (B, T, C), x.shape

    any_jmp = bool((x[..., OPC_JMP] > 0.5).any())
    any_bz = bool((x[..., OPC_BZ] > 0.5).any())
    any_bnz = bool((x[..., OPC_BNZ] > 0.5).any())
    if not (any_jmp or any_bz or any_bnz):
        return x.copy()

    xr = x.reshape(N_CORES, ROWS_PER_CORE, C)
    if any_jmp:
        nc = _get_kernel("jmp")
        a = np.ascontiguousarray(xr[:, :, IMM0:IMM0 + 8])
        in_maps = [{"a": a[c]} for c in range(N_CORES)]
        res = _run_spmd(nc, in_maps)

        out = x.copy()
        orows = out.reshape(N_CORES, ROWS_PER_CORE, C)
        nt = CONFIG["tiles"]
        wt = W // nt
        for c in range(N_CORES):
            nib = res.results[c]["onib"]          # [nt, P, 8*wt] u16
            nib = nib.reshape(nt, P, 4, wt, 2)    # [t, p, plane, w, half]
            # row = (t*P + p)*wt + w ; col = 4*half + plane
            nib = nib.transpose(0, 1, 3, 4, 2).reshape(ROWS_PER_CORE, 8)
            orows[c, :, PC0:PC0 + 8] = nib.astype(np.float32)
        orows[:, :, BT] = 1.0
        return out

    nc = _get_kernel("patch")
    patch = _host_patch(x).reshape(N_CORES, ROWS_PER_CORE, 9)
    in_maps = [{"x": xr[c], "patch": patch[c]} for c in range(N_CORES)]
    res = _run_spmd(nc, in_maps)
    out = np.empty((N_CORES, ROWS_PER_CORE, C), dtype=np.float32)
    for c in range(N_CORES):
        out[c] = res.results[c]["out"]
    return out.reshape(B, T, C)


# revision 10
# speedup vs baseline: 1.0603x; 1.0603x over previous
"""Trainium2 Bass kernel for nn_ControlFlowExpert_62380105007397.

Reference semantics (CPU-XLA eager jax):
  x: [16, 8192, 208] fp32.
  imm = sequential fp32 chain sum_n x[..., 195+n] * 16^n   (n = 0..7)
  pc  = same over cols 171..178
  ax  = int32-wrap sum of trunc-toward-zero casts of cols 163..170 times 16^n
  any_jmp/any_bz/any_bnz = global any() of opcode cols 90/92/93 > 0.5
  If any flag set: out = x with cols 171..178 = nibbles of int32(new_pc)
  and col 203 = branch-taken flag; else out = x.

Strategy: the op reads only 8 input columns and modifies only 9 output
columns; everything else is identity. Flags are computed on host (3
column scans) and select a compile-time specialized device kernel. The
dominant any_jmp path: host slices the 8 imm columns into a compact
[rows, 8] fp32 array per core (batch-sharded over 8 cores); the device
computes the exact sequential fp32 chain (bit-identical to XLA CPU),
truncates toward zero (RNE cast + sign-bit-or correction, HW-verified
exact), and extracts all 8 nibbles with 4 uint16-bitcast shift-and ops,
writing compact nibble planes. Host assembles out = x.copy() with the
device nibbles spliced into cols 171..178 and col 203 = 1. Device HBM
traffic drops 37x vs streaming all 208 columns. Rare paths (bz/bnz
without jmp) use a host-computed 9-column patch spliced on device.
"""

import sys

if "/opt/trn_rl_repo" not in sys.path:
    sys.path.insert(0, "/opt/trn_rl_repo")

import numpy as np

B, T, C = 16, 8192, 208
N_CORES = 8
ROWS_PER_CORE = (B // N_CORES) * T          # 16384
P = 128                                     # SBUF partitions
W = ROWS_PER_CORE // P                      # 128 rows per partition

OPC_JMP, OPC_BZ, OPC_BNZ = 90, 92, 93
AX0, PC0, IMM0, BT = 163, 171, 195, 203

_kernel_cache = {}

# perf knobs (test harness overrides these before first kernel() call)
CONFIG = {"tiles": 2, "bufs": 2, "out_engine": "scalar", "impl": "raw",
          "no_out_wait": True}


def _emit_compact(nc, mybir, pool, sp, a3, nibt, tag):
    """DVE pipeline: a3 [P, Wt, 8] fp32 view -> nibt [P, 4, 2*Wt] u16 view.

    chain (7 STT, exact sequential fp32 order) -> trunc toward zero
    (9 ops, HW-verified) -> 4 shift-and nibble ops on the u16 bitcast.
    """
    A = mybir.AluOpType
    f32, i32, u32, u16 = (mybir.dt.float32, mybir.dt.int32, mybir.dt.uint32,
                          mybir.dt.uint16)
    ws = a3.shape[1]

    acc = sp.tile([P, ws], f32, tag=f"acc{tag}")
    nc.vector.scalar_tensor_tensor(
        out=acc[:], in0=a3[:, :, 1], scalar=16.0,
        in1=a3[:, :, 0], op0=A.mult, op1=A.add)
    for n in range(2, 8):
        nacc = sp.tile([P, ws], f32, tag=f"acc{n}{tag}")
        nc.vector.scalar_tensor_tensor(
            out=nacc[:], in0=a3[:, :, n], scalar=float(16.0 ** n),
            in1=acc[:], op0=A.mult, op1=A.add)
        acc = nacc

    # trunc toward zero: y = rne(acc); fy = f(y); d = acc - fy; w = d*fy;
    # corr = or(bits(w<0), signbit(fy)); ft = fy - corr; v = i32(ft)
    y = sp.tile([P, ws], i32, tag=f"y{tag}")
    nc.vector.tensor_copy(out=y[:], in_=acc[:])
    fy = sp.tile([P, ws], f32, tag=f"fy{tag}")
    nc.vector.tensor_copy(out=fy[:], in_=y[:])
    d = sp.tile([P, ws], f32, tag=f"d{tag}")
    nc.vector.scalar_tensor_tensor(
        out=d[:], in0=fy[:], scalar=-1.0, in1=acc[:], op0=A.mult, op1=A.add)
    w = sp.tile([P, ws], f32, tag=f"w{tag}")
    nc.vector.tensor_tensor(out=w[:], in0=d[:], in1=fy[:], op=A.mult)
    mask = sp.tile([P, ws], f32, tag=f"mask{tag}")
    nc.vector.tensor_scalar(out=mask[:], in0=w[:], scalar1=0.0, scalar2=None,
                            op0=A.is_lt)
    sb = sp.tile([P, ws], u32, tag=f"sb{tag}")
    nc.vector.tensor_scalar(out=sb[:], in0=fy[:].bitcast(u32),
                            scalar1=0x80000000, scalar2=None,
                            op0=A.bitwise_and)
    corr = sp.tile([P, ws], u32, tag=f"corr{tag}")
    nc.vector.tensor_tensor(out=corr[:], in0=mask[:].bitcast(u32),
                            in1=sb[:], op=A.bitwise_or)
    ft = sp.tile([P, ws], f32, tag=f"ft{tag}")
    nc.vector.tensor_tensor(out=ft[:], in0=fy[:], in1=corr[:].bitcast(f32),
                            op=A.subtract)
    v = sp.tile([P, ws], i32, tag=f"v{tag}")
    nc.vector.tensor_copy(out=v[:], in_=ft[:])

    # nibbles: u16 view [P, 2*ws]: elem (2w+j) = halfword j of row w.
    # plane n holds nibble n (j=0) and nibble n+4 (j=1) of each row.
    v16 = v[:].bitcast(u16)
    nc.vector.tensor_scalar(out=nibt[:, 0, :], in0=v16, scalar1=15,
                            scalar2=None, op0=A.bitwise_and)
    for n in range(1, 4):
        nc.vector.tensor_scalar(out=nibt[:, n, :], in0=v16,
                                scalar1=4 * n, scalar2=15,
                                op0=A.logical_shift_right,
                                op1=A.bitwise_and)


def _build_jmp_compact():
    """any_jmp path: compact columns in, nibble planes out."""
    import concourse.bacc as bacc
    import concourse.mybir as mybir
    from concourse.tile import TileContext

    f32 = mybir.dt.float32
    u16 = mybir.dt.uint16
    nt = CONFIG["tiles"]
    wt = W // nt                    # rows per partition per tile
    tile_rows = P * wt

    nc = bacc.Bacc("TRN2")
    out_eng = getattr(nc, CONFIG["out_engine"])
    a = nc.dram_tensor("a", [ROWS_PER_CORE, 8], f32, kind="ExternalInput")
    onib = nc.dram_tensor("onib", [nt, P, 8 * wt], u16, kind="ExternalOutput")

    with TileContext(nc) as tc:
        with tc.tile_pool(name="sbuf", bufs=CONFIG["bufs"]) as pool, \
             tc.tile_pool(name="small", bufs=2) as sp:
            for t in range(nt):
                rows = slice(t * tile_rows, (t + 1) * tile_rows)
                at = pool.tile([P, wt * 8], f32, tag="at")
                nc.sync.dma_start(
                    out=at[:],
                    in_=a[rows, :].rearrange("(p w) c -> p (w c)", p=P))
                a3 = at[:].rearrange("p (w c) -> p w c", c=8)
                nibt = pool.tile([P, 8 * wt], u16, tag="nibt")
                n3 = nibt[:].rearrange("p (n w) -> p n w", n=4)
                _emit_compact(nc, mybir, pool, sp, a3, n3, tag="")
                out_eng.dma_start(out=onib[t], in_=nibt[:])
    nc.finalize()
    return nc


def _build_jmp_raw():
    """any_jmp path, raw bass: no TileContext epilogue, manual sems,
    two interleaved row-streams so RAW-dependent DVE ops never wait on
    pipeline drains (the other stream's op provides the spacing).

    Input is host-transposed to [stream, P, 8, ws] so every chain read is
    contiguous (strided column reads cost ~2x on the DVE read port). The
    two in-DMAs issue in parallel from sync and scalar. sem_cmp incs ride
    the last nibble op of each stream (@complete => committed)."""
    from contextlib import ExitStack

    import concourse.bacc as bacc
    import concourse.mybir as mybir

    A = mybir.AluOpType
    f32, i32, u32, u16 = (mybir.dt.float32, mybir.dt.int32, mybir.dt.uint32,
                          mybir.dt.uint16)
    NS = CONFIG["tiles"]            # streams
    ws = W // NS                    # rows per partition per stream

    nc = bacc.Bacc("TRN2")
    a = nc.dram_tensor("a", [NS, P, 8 * ws], f32, kind="ExternalInput")
    onib = nc.dram_tensor("onib", [NS, P, 8 * ws], u16, kind="ExternalOutput")

    with ExitStack() as st:
        ats = [st.enter_context(nc.sbuf_tensor(f"at{s}", [P, 8 * ws], f32))
               for s in range(NS)]
        nibs = [st.enter_context(nc.sbuf_tensor(f"nib{s}", [P, 8 * ws], u16))
                for s in range(NS)]
        tmp = []
        for s in range(NS):
            t = {}
            for k in ("accA", "accB", "fy", "d", "w", "mask", "ft"):
                t[k] = st.enter_context(
                    nc.sbuf_tensor(f"t{s}_{k}", [P, ws], f32))
            for k in ("y", "v"):
                t[k] = st.enter_context(
                    nc.sbuf_tensor(f"t{s}_{k}", [P, ws], i32))
            for k in ("sb", "corr"):
                t[k] = st.enter_context(
                    nc.sbuf_tensor(f"t{s}_{k}", [P, ws], u32))
            tmp.append(t)
        sems_in = [st.enter_context(nc.semaphore(f"sin{s}"))
                   for s in range(NS)]
        sem_cmp = st.enter_context(nc.semaphore("scmp"))
        sem_out = st.enter_context(nc.semaphore("sout"))
        block = st.enter_context(nc.Block())

        @block.sync
        def _(sync):
            sync.dma_start(ats[0][:], a[0]).then_inc(sems_in[0], 16)

        def steps(s):
            """Yield the 20-op DVE program for stream s as thunks."""
            t = tmp[s]
            a2 = ats[s][:].rearrange("p (c w) -> p c w", c=8)
            acc_cur, acc_nxt = t["accA"], t["accB"]

            def op_chain0(acc_cur=acc_cur):
                return nc.vector.scalar_tensor_tensor(
                    out=acc_cur[:], in0=a2[:, 1, :], scalar=16.0,
                    in1=a2[:, 0, :], op0=A.mult, op1=A.add)
            yield op_chain0
            for n in range(2, 8):
                def op_chainN(n=n, dst=acc_nxt, src=acc_cur):
                    return nc.vector.scalar_tensor_tensor(
                        out=dst[:], in0=a2[:, n, :], scalar=float(16.0 ** n),
                        in1=src[:], op0=A.mult, op1=A.add)
                yield op_chainN
                acc_cur, acc_nxt = acc_nxt, acc_cur
            acc = acc_cur
            yield lambda: nc.vector.tensor_copy(out=t["y"][:], in_=acc[:])
            yield lambda: nc.vector.tensor_copy(out=t["fy"][:], in_=t["y"][:])
            yield lambda: nc.vector.scalar_tensor_tensor(
                out=t["d"][:], in0=t["fy"][:], scalar=-1.0, in1=acc[:],
                op0=A.mult, op1=A.add)
            yield lambda: nc.vector.tensor_tensor(
                out=t["w"][:], in0=t["d"][:], in1=t["fy"][:], op=A.mult)
            yield lambda: nc.vector.tensor_scalar(
                out=t["mask"][:], in0=t["w"][:], scalar1=0.0, scalar2=None,
                op0=A.is_lt)
            yield lambda: nc.vector.tensor_scalar(
                out=t["sb"][:], in0=t["fy"][:].bitcast(u32),
                scalar1=0x80000000, scalar2=None, op0=A.bitwise_and)
            yield lambda: nc.vector.tensor_tensor(
                out=t["corr"][:], in0=t["mask"][:].bitcast(u32),
                in1=t["sb"][:], op=A.bitwise_or)
            yield lambda: nc.vector.tensor_tensor(
                out=t["ft"][:], in0=t["fy"][:], in1=t["corr"][:].bitcast(f32),
                op=A.subtract)
            yield lambda: nc.vector.tensor_copy(out=t["v"][:], in_=t["ft"][:])
            v16 = t["v"][:].bitcast(u16)
            n3 = nibs[s][:].rearrange("p (n w) -> p n w", n=4)

            def op_nib0():
                return nc.vector.tensor_scalar(
                    out=n3[:, 0, :], in0=v16, scalar1=15, scalar2=None,
                    op0=A.bitwise_and)
            yield op_nib0
            for n in range(1, 4):
                def op_nibN(n=n):
                    return nc.vector.tensor_scalar(
                        out=n3[:, n, :], in0=v16, scalar1=4 * n, scalar2=15,
                        op0=A.logical_shift_right, op1=A.bitwise_and)
                yield op_nibN

        @block.vector
        def _(vector):
            gens = [steps(s) for s in range(NS)]
            started = [False] * NS
            done = [False] * NS
            lasts = [None] * NS
            while not all(done):
                for s in range(NS):
                    if done[s]:
                        continue
                    if not started[s]:
                        vector.wait_ge(sems_in[s], 16)
                        started[s] = True
                    try:
                        lasts[s] = next(gens[s])()
                    except StopIteration:
                        done[s] = True
                        # @complete fires after the op's writes commit, so
                        # the out-DMA reads consistent SBUF
                        lasts[s].then_inc(sem_cmp, 1)

        @block.scalar
        def _(scalar):
            if NS > 1:
                scalar.dma_start(ats[1][:], a[1]).then_inc(sems_in[1], 16)
            for s in range(NS):
                scalar.wait_ge(sem_cmp, s + 1)
                scalar.dma_start(onib[s], nibs[s][:]).then_inc(sem_out, 16)
            if not CONFIG.get("no_out_wait"):
                scalar.wait_ge(sem_out, 16 * NS)

    nc.finalize()
    return nc


def _build_patch_kernel():
    """Device kernel for rare flag combos: stream x, splice host patch."""
    import concourse.bacc as bacc
    import concourse.mybir as mybir
    from concourse.tile import TileContext

    f32 = mybir.dt.float32
    W16 = 16
    TILE_ROWS = P * W16
    N_TILES = ROWS_PER_CORE // TILE_ROWS

    nc = bacc.Bacc("TRN2")
    x = nc.dram_tensor("x", [ROWS_PER_CORE, C], f32, kind="ExternalInput")
    patch = nc.dram_tensor("patch", [ROWS_PER_CORE, 9], f32, kind="ExternalInput")
    out = nc.dram_tensor("out", [ROWS_PER_CORE, C], f32, kind="ExternalOutput")

    with TileContext(nc) as tc:
        with tc.tile_pool(name="sbuf", bufs=4) as pool, \
             tc.tile_pool(name="small", bufs=3) as sp:
            for t in range(N_TILES):
                rows = slice(t * TILE_ROWS, (t + 1) * TILE_ROWS)
                xt = pool.tile([P, W16 * C], f32, tag="xt")
                x3 = xt[:].rearrange("p (w c) -> p w c", c=C)
                nc.sync.dma_start(
                    out=xt[:],
                    in_=x[rows, :].rearrange("(p w) c -> p (w c)", p=P))
                pt = sp.tile([P, W16 * 9], f32, tag="pt")
                p3 = pt[:].rearrange("p (w c) -> p w c", c=9)
                nc.sync.dma_start(
                    out=pt[:],
                    in_=patch[rows, :].rearrange("(p w) c -> p (w c)", p=P))
                nc.vector.tensor_copy(out=x3[:, :, PC0:PC0 + 8], in_=p3[:, :, 0:8])
                nc.vector.tensor_copy(out=x3[:, :, BT], in_=p3[:, :, 8])
                nc.sync.dma_start(
                    out=out[rows, :].rearrange("(p w) c -> p (w c)", p=P),
                    in_=xt[:])
    nc.finalize()
    return nc


def _get_kernel(name):
    if name not in _kernel_cache:
        if name == "jmp":
            builder = (_build_jmp_raw if CONFIG.get("impl") == "raw"
                       else _build_jmp_compact)
            _kernel_cache[name] = builder()
        else:
            _kernel_cache[name] = _build_patch_kernel()
    return _kernel_cache[name]


# test.py can set _RUN_KWARGS["trace"] = True and read LAST for profiling.
_RUN_KWARGS = {}
LAST = None


def _run_spmd(nc, in_maps):
    global LAST
    from concourse.bass_utils import run_bass_kernel_spmd
    LAST = run_bass_kernel_spmd(nc, in_maps, core_ids=list(range(N_CORES)),
                                **_RUN_KWARGS)
    return LAST


def _host_patch(x):
    """Exact CPU-XLA-equivalent computation of the 9 modified columns."""
    pw = np.float32(16.0) ** np.arange(8, dtype=np.float32)
    imm = x[..., IMM0].astype(np.float32)
    pc = x[..., PC0].astype(np.float32)
    for n in range(1, 8):
        imm = (x[..., IMM0 + n] * pw[n] + imm).astype(np.float32)
        pc = (x[..., PC0 + n] * pw[n] + pc).astype(np.float32)
    axs = np.zeros(x.shape[:-1], dtype=np.int64)
    for n in range(8):
        axs += x[..., AX0 + n].astype(np.int32).astype(np.int64) * (16 ** n)
    ax = ((axs + 2**31) % 2**32 - 2**31).astype(np.int32)
    ax_is_zero = ax == 0

    any_bz = bool((x[..., OPC_BZ] > 0.5).any())

    pc8 = (pc + np.float32(8.0)).astype(np.float32)
    if any_bz:
        new_pc = np.where(ax_is_zero, imm, pc8)
        bt = ax_is_zero.astype(np.float32)
    else:  # any_bnz
        new_pc = np.where(~ax_is_zero, imm, pc8)
        bt = (~ax_is_zero).astype(np.float32)
    v = new_pc.astype(np.int32)
    shifts = np.arange(8, dtype=np.int32) * 4
    nibs = ((v[..., None] >> shifts) & 15).astype(np.float32)
    return np.concatenate([nibs, bt[..., None]], axis=-1)


def kernel(x):
    x = np.ascontiguousarray(np.asarray(x), dtype=np.float32)
    assert x.shape == (B, T, C), x.shape

    any_jmp = bool((x[..., OPC_JMP] > 0.5).any())
    any_bz = bool((x[..., OPC_BZ] > 0.5).any())
    any_bnz = bool((x[..., OPC_BNZ] > 0.5).any())
    if not (any_jmp or any_bz or any_bnz):
        return x.copy()

    xr = x.reshape(N_CORES, ROWS_PER_CORE, C)
    if any_jmp:
        nc = _get_kernel("jmp")
        nt = CONFIG["tiles"]
        wt = W // nt
        if CONFIG.get("impl") == "raw":
            # transposed layout: a[core][s, p, n, w] = row (s*P + p)*wt + w,
            # col IMM0 + n  -> every chain read on device is contiguous
            a = xr[:, :, IMM0:IMM0 + 8].reshape(N_CORES, nt, P, wt, 8)
            a = np.ascontiguousarray(a.transpose(0, 1, 2, 4, 3)).reshape(
                N_CORES, nt, P, 8 * wt)
        else:
            a = np.ascontiguousarray(xr[:, :, IMM0:IMM0 + 8])
        in_maps = [{"a": a[c]} for c in range(N_CORES)]
        res = _run_spmd(nc, in_maps)

        out = x.copy()
        orows = out.reshape(N_CORES, ROWS_PER_CORE, C)
        nt = CONFIG["tiles"]
        wt = W // nt
        for c in range(N_CORES):
            nib = res.results[c]["onib"]          # [nt, P, 8*wt] u16
            nib = nib.reshape(nt, P, 4, wt, 2)    # [t, p, plane, w, half]
            # row = (t*P + p)*wt + w ; col = 4*half + plane
            nib = nib.transpose(0, 1, 3, 4, 2).reshape(ROWS_PER_CORE, 8)
            orows[c, :, PC0:PC0 + 8] = nib.astype(np.float32)
        orows[:, :, BT] = 1.0
        return out

    nc = _get_kernel("patch")
    patch = _host_patch(x).reshape(N_CORES, ROWS_PER_CORE, 9)
    in_maps = [{"x": xr[c], "patch": patch[c]} for c in range(N_CORES)]
    res = _run_spmd(nc, in_maps)
    out = np.empty((N_CORES, ROWS_PER_CORE, C), dtype=np.float32)
    for c in range(N_CORES):
        out[c] = res.results[c]["out"]
    return out.reshape(B, T, C)


# revision 13
# speedup vs baseline: 1.1411x; 1.0762x over previous
"""Trainium2 Bass kernel for nn_ControlFlowExpert_62380105007397.

Reference semantics (CPU-XLA eager jax):
  x: [16, 8192, 208] fp32.
  imm = sequential fp32 chain sum_n x[..., 195+n] * 16^n   (n = 0..7)
  pc  = same over cols 171..178
  ax  = int32-wrap sum of trunc-toward-zero casts of cols 163..170 times 16^n
  any_jmp/any_bz/any_bnz = global any() of opcode cols 90/92/93 > 0.5
  If any flag set: out = x with cols 171..178 = nibbles of int32(new_pc)
  and col 203 = branch-taken flag; else out = x.

Strategy: the op reads only 8 input columns and modifies only 9 output
columns; everything else is identity. Flags are computed on host (3
column scans) and select a compile-time specialized device kernel. The
dominant any_jmp path: host slices the 8 imm columns into a compact
[rows, 8] fp32 array per core (batch-sharded over 8 cores); the device
computes the exact sequential fp32 chain (bit-identical to XLA CPU),
truncates toward zero (RNE cast + sign-bit-or correction, HW-verified
exact), and extracts all 8 nibbles with 4 uint16-bitcast shift-and ops,
writing compact nibble planes. Host assembles out = x.copy() with the
device nibbles spliced into cols 171..178 and col 203 = 1. Device HBM
traffic drops 37x vs streaming all 208 columns. Rare paths (bz/bnz
without jmp) use a host-computed 9-column patch spliced on device.
"""

import sys

if "/opt/trn_rl_repo" not in sys.path:
    sys.path.insert(0, "/opt/trn_rl_repo")

import numpy as np

B, T, C = 16, 8192, 208
N_CORES = 8
ROWS_PER_CORE = (B // N_CORES) * T          # 16384
P = 128                                     # SBUF partitions
W = ROWS_PER_CORE // P                      # 128 rows per partition

OPC_JMP, OPC_BZ, OPC_BNZ = 90, 92, 93
AX0, PC0, IMM0, BT = 163, 171, 195, 203

_kernel_cache = {}

# perf knobs (test harness overrides these before first kernel() call)
CONFIG = {"tiles": 2, "bufs": 2, "out_engine": "scalar", "impl": "raw",
          "no_out_wait": True}


def _emit_compact(nc, mybir, pool, sp, a3, nibt, tag):
    """DVE pipeline: a3 [P, Wt, 8] fp32 view -> nibt [P, 4, 2*Wt] u16 view.

    chain (7 STT, exact sequential fp32 order) -> trunc toward zero
    (9 ops, HW-verified) -> 4 shift-and nibble ops on the u16 bitcast.
    """
    A = mybir.AluOpType
    f32, i32, u32, u16 = (mybir.dt.float32, mybir.dt.int32, mybir.dt.uint32,
                          mybir.dt.uint16)
    ws = a3.shape[1]

    acc = sp.tile([P, ws], f32, tag=f"acc{tag}")
    nc.vector.scalar_tensor_tensor(
        out=acc[:], in0=a3[:, :, 1], scalar=16.0,
        in1=a3[:, :, 0], op0=A.mult, op1=A.add)
    for n in range(2, 8):
        nacc = sp.tile([P, ws], f32, tag=f"acc{n}{tag}")
        nc.vector.scalar_tensor_tensor(
            out=nacc[:], in0=a3[:, :, n], scalar=float(16.0 ** n),
            in1=acc[:], op0=A.mult, op1=A.add)
        acc = nacc

    # trunc toward zero: y = rne(acc); fy = f(y); d = acc - fy; w = d*fy;
    # corr = or(bits(w<0), signbit(fy)); ft = fy - corr; v = i32(ft)
    y = sp.tile([P, ws], i32, tag=f"y{tag}")
    nc.vector.tensor_copy(out=y[:], in_=acc[:])
    fy = sp.tile([P, ws], f32, tag=f"fy{tag}")
    nc.vector.tensor_copy(out=fy[:], in_=y[:])
    d = sp.tile([P, ws], f32, tag=f"d{tag}")
    nc.vector.scalar_tensor_tensor(
        out=d[:], in0=fy[:], scalar=-1.0, in1=acc[:], op0=A.mult, op1=A.add)
    w = sp.tile([P, ws], f32, tag=f"w{tag}")
    nc.vector.tensor_tensor(out=w[:], in0=d[:], in1=fy[:], op=A.mult)
    mask = sp.tile([P, ws], f32, tag=f"mask{tag}")
    nc.vector.tensor_scalar(out=mask[:], in0=w[:], scalar1=0.0, scalar2=None,
                            op0=A.is_lt)
    sb = sp.tile([P, ws], u32, tag=f"sb{tag}")
    nc.vector.tensor_scalar(out=sb[:], in0=fy[:].bitcast(u32),
                            scalar1=0x80000000, scalar2=None,
                            op0=A.bitwise_and)
    corr = sp.tile([P, ws], u32, tag=f"corr{tag}")
    nc.vector.tensor_tensor(out=corr[:], in0=mask[:].bitcast(u32),
                            in1=sb[:], op=A.bitwise_or)
    ft = sp.tile([P, ws], f32, tag=f"ft{tag}")
    nc.vector.tensor_tensor(out=ft[:], in0=fy[:], in1=corr[:].bitcast(f32),
                            op=A.subtract)
    v = sp.tile([P, ws], i32, tag=f"v{tag}")
    nc.vector.tensor_copy(out=v[:], in_=ft[:])

    # nibbles: u16 view [P, 2*ws]: elem (2w+j) = halfword j of row w.
    # plane n holds nibble n (j=0) and nibble n+4 (j=1) of each row.
    v16 = v[:].bitcast(u16)
    nc.vector.tensor_scalar(out=nibt[:, 0, :], in0=v16, scalar1=15,
                            scalar2=None, op0=A.bitwise_and)
    for n in range(1, 4):
        nc.vector.tensor_scalar(out=nibt[:, n, :], in0=v16,
                                scalar1=4 * n, scalar2=15,
                                op0=A.logical_shift_right,
                                op1=A.bitwise_and)


def _build_jmp_compact():
    """any_jmp path: compact columns in, nibble planes out."""
    import concourse.bacc as bacc
    import concourse.mybir as mybir
    from concourse.tile import TileContext

    f32 = mybir.dt.float32
    u16 = mybir.dt.uint16
    nt = CONFIG["tiles"]
    wt = W // nt                    # rows per partition per tile
    tile_rows = P * wt

    nc = bacc.Bacc("TRN2")
    out_eng = getattr(nc, CONFIG["out_engine"])
    a = nc.dram_tensor("a", [ROWS_PER_CORE, 8], f32, kind="ExternalInput")
    onib = nc.dram_tensor("onib", [nt, P, 8 * wt], u16, kind="ExternalOutput")

    with TileContext(nc) as tc:
        with tc.tile_pool(name="sbuf", bufs=CONFIG["bufs"]) as pool, \
             tc.tile_pool(name="small", bufs=2) as sp:
            for t in range(nt):
                rows = slice(t * tile_rows, (t + 1) * tile_rows)
                at = pool.tile([P, wt * 8], f32, tag="at")
                nc.sync.dma_start(
                    out=at[:],
                    in_=a[rows, :].rearrange("(p w) c -> p (w c)", p=P))
                a3 = at[:].rearrange("p (w c) -> p w c", c=8)
                nibt = pool.tile([P, 8 * wt], u16, tag="nibt")
                n3 = nibt[:].rearrange("p (n w) -> p n w", n=4)
                _emit_compact(nc, mybir, pool, sp, a3, n3, tag="")
                out_eng.dma_start(out=onib[t], in_=nibt[:])
    nc.finalize()
    return nc


def _build_jmp_raw():
    """any_jmp path, raw bass: no TileContext epilogue, manual sems,
    two interleaved row-streams so RAW-dependent DVE ops never wait on
    pipeline drains (the other stream's op provides the spacing).

    Input is host-transposed to [stream, P, 8, ws] so every chain read is
    contiguous (strided column reads cost ~2x on the DVE read port). The
    two in-DMAs issue in parallel from sync and scalar. sem_cmp incs ride
    the last nibble op of each stream (@complete => committed)."""
    from contextlib import ExitStack

    import concourse.bacc as bacc
    import concourse.mybir as mybir

    A = mybir.AluOpType
    f32, i32, u32, u16 = (mybir.dt.float32, mybir.dt.int32, mybir.dt.uint32,
                          mybir.dt.uint16)
    NS = CONFIG["tiles"]            # streams
    ws = W // NS                    # rows per partition per stream

    nc = bacc.Bacc("TRN2")
    a = nc.dram_tensor("a", [NS, P, 8 * ws], f32, kind="ExternalInput")
    onib = nc.dram_tensor("onib", [NS, P, 8 * ws], u16, kind="ExternalOutput")

    with ExitStack() as st:
        ats = [st.enter_context(nc.sbuf_tensor(f"at{s}", [P, 8 * ws], f32))
               for s in range(NS)]
        nibs = [st.enter_context(nc.sbuf_tensor(f"nib{s}", [P, 8 * ws], u16))
                for s in range(NS)]
        tmp = []
        for s in range(NS):
            t = {}
            for k in ("accA", "accB", "fy", "d", "w", "mask", "ft"):
                t[k] = st.enter_context(
                    nc.sbuf_tensor(f"t{s}_{k}", [P, ws], f32))
            for k in ("y", "v"):
                t[k] = st.enter_context(
                    nc.sbuf_tensor(f"t{s}_{k}", [P, ws], i32))
            for k in ("sb", "corr"):
                t[k] = st.enter_context(
                    nc.sbuf_tensor(f"t{s}_{k}", [P, ws], u32))
            tmp.append(t)
        sem_in = st.enter_context(nc.semaphore("sin"))
        sem_cmp = st.enter_context(nc.semaphore("scmp"))
        sem_out = st.enter_context(nc.semaphore("sout"))
        block = st.enter_context(nc.Block())

        h = 4 * ws  # half-tile: columns 0-3 / 4-7

        @block.sync
        def _(sync):
            # 4 sequential quarter-DMAs (HWDGE FIFO per engine => in-order
            # completion, one counting sem): s0 cols0-3, s1 cols0-3,
            # s0 cols4-7, s1 cols4-7. Compute starts after the first.
            sync.dma_start(ats[0][:, 0:h], a[0][:, 0:h]).then_inc(sem_in, 16)
            if NS > 1:
                sync.dma_start(ats[1][:, 0:h],
                               a[1][:, 0:h]).then_inc(sem_in, 16)
            sync.dma_start(ats[0][:, h:2 * h],
                           a[0][:, h:2 * h]).then_inc(sem_in, 16)
            if NS > 1:
                sync.dma_start(ats[1][:, h:2 * h],
                               a[1][:, h:2 * h]).then_inc(sem_in, 16)
            # outputs, gated on each stream's completion inc
            for s in range(NS):
                sync.wait_ge(sem_cmp, s + 1)
                sync.dma_start(onib[s], nibs[s][:]).then_inc(sem_out, 16)
            if not CONFIG.get("no_out_wait"):
                sync.wait_ge(sem_out, 16 * NS)

        def steps(s):
            """Yield the 20-op DVE program for stream s as thunks."""
            t = tmp[s]
            a2 = ats[s][:].rearrange("p (c w) -> p c w", c=8)
            acc_cur, acc_nxt = t["accA"], t["accB"]

            def op_chain0(acc_cur=acc_cur):
                return nc.vector.scalar_tensor_tensor(
                    out=acc_cur[:], in0=a2[:, 1, :], scalar=16.0,
                    in1=a2[:, 0, :], op0=A.mult, op1=A.add)
            yield 16 * (s + 1), op_chain0
            for n in range(2, 8):
                def op_chainN(n=n, dst=acc_nxt, src=acc_cur):
                    return nc.vector.scalar_tensor_tensor(
                        out=dst[:], in0=a2[:, n, :], scalar=float(16.0 ** n),
                        in1=src[:], op0=A.mult, op1=A.add)
                # col 4 (n=4) sits in the second quarter-DMA of this stream
                yield (16 * (NS + s + 1) if n == 4 else None), op_chainN
                acc_cur, acc_nxt = acc_nxt, acc_cur
            acc = acc_cur
            yield None, lambda: nc.vector.tensor_copy(out=t["y"][:], in_=acc[:])
            yield None, lambda: nc.vector.tensor_copy(out=t["fy"][:], in_=t["y"][:])
            yield None, lambda: nc.vector.scalar_tensor_tensor(
                out=t["d"][:], in0=t["fy"][:], scalar=-1.0, in1=acc[:],
                op0=A.mult, op1=A.add)
            yield None, lambda: nc.vector.tensor_tensor(
                out=t["w"][:], in0=t["d"][:], in1=t["fy"][:], op=A.mult)
            yield None, lambda: nc.vector.tensor_scalar(
                out=t["mask"][:], in0=t["w"][:], scalar1=0.0, scalar2=None,
                op0=A.is_lt)
            yield None, lambda: nc.vector.tensor_scalar(
                out=t["sb"][:], in0=t["fy"][:].bitcast(u32),
                scalar1=0x80000000, scalar2=None, op0=A.bitwise_and)
            yield None, lambda: nc.vector.tensor_tensor(
                out=t["corr"][:], in0=t["mask"][:].bitcast(u32),
                in1=t["sb"][:], op=A.bitwise_or)
            yield None, lambda: nc.vector.tensor_tensor(
                out=t["ft"][:], in0=t["fy"][:], in1=t["corr"][:].bitcast(f32),
                op=A.subtract)
            yield None, lambda: nc.vector.tensor_copy(out=t["v"][:], in_=t["ft"][:])
            v16 = t["v"][:].bitcast(u16)
            n3 = nibs[s][:].rearrange("p (n w) -> p n w", n=4)

            def op_nib0():
                return nc.vector.tensor_scalar(
                    out=n3[:, 0, :], in0=v16, scalar1=15, scalar2=None,
                    op0=A.bitwise_and)
            yield None, op_nib0
            for n in range(1, 4):
                def op_nibN(n=n):
                    return nc.vector.tensor_scalar(
                        out=n3[:, n, :], in0=v16, scalar1=4 * n, scalar2=15,
                        op0=A.logical_shift_right, op1=A.bitwise_and)
                yield None, op_nibN

        @block.vector
        def _(vector):
            gens = [steps(s) for s in range(NS)]
            done = [False] * NS
            lasts = [None] * NS
            while not all(done):
                for s in range(NS):
                    if done[s]:
                        continue
                    try:
                        wait_val, thunk = next(gens[s])
                    except StopIteration:
                        done[s] = True
                        # @complete fires after the op's writes commit, so
                        # the out-DMA reads consistent SBUF
                        lasts[s].then_inc(sem_cmp, 1)
                        continue
                    if wait_val is not None:
                        vector.wait_ge(sem_in, wait_val)
                    lasts[s] = thunk()

    nc.finalize()
    return nc


def _build_patch_kernel():
    """Device kernel for rare flag combos: stream x, splice host patch."""
    import concourse.bacc as bacc
    import concourse.mybir as mybir
    from concourse.tile import TileContext

    f32 = mybir.dt.float32
    W16 = 16
    TILE_ROWS = P * W16
    N_TILES = ROWS_PER_CORE // TILE_ROWS

    nc = bacc.Bacc("TRN2")
    x = nc.dram_tensor("x", [ROWS_PER_CORE, C], f32, kind="ExternalInput")
    patch = nc.dram_tensor("patch", [ROWS_PER_CORE, 9], f32, kind="ExternalInput")
    out = nc.dram_tensor("out", [ROWS_PER_CORE, C], f32, kind="ExternalOutput")

    with TileContext(nc) as tc:
        with tc.tile_pool(name="sbuf", bufs=4) as pool, \
             tc.tile_pool(name="small", bufs=3) as sp:
            for t in range(N_TILES):
                rows = slice(t * TILE_ROWS, (t + 1) * TILE_ROWS)
                xt = pool.tile([P, W16 * C], f32, tag="xt")
                x3 = xt[:].rearrange("p (w c) -> p w c", c=C)
                nc.sync.dma_start(
                    out=xt[:],
                    in_=x[rows, :].rearrange("(p w) c -> p (w c)", p=P))
                pt = sp.tile([P, W16 * 9], f32, tag="pt")
                p3 = pt[:].rearrange("p (w c) -> p w c", c=9)
                nc.sync.dma_start(
                    out=pt[:],
                    in_=patch[rows, :].rearrange("(p w) c -> p (w c)", p=P))
                nc.vector.tensor_copy(out=x3[:, :, PC0:PC0 + 8], in_=p3[:, :, 0:8])
                nc.vector.tensor_copy(out=x3[:, :, BT], in_=p3[:, :, 8])
                nc.sync.dma_start(
                    out=out[rows, :].rearrange("(p w) c -> p (w c)", p=P),
                    in_=xt[:])
    nc.finalize()
    return nc


def _get_kernel(name):
    if name not in _kernel_cache:
        if name == "jmp":
            builder = (_build_jmp_raw if CONFIG.get("impl") == "raw"
                       else _build_jmp_compact)
            _kernel_cache[name] = builder()
        else:
            _kernel_cache[name] = _build_patch_kernel()
    return _kernel_cache[name]


# test.py can set _RUN_KWARGS["trace"] = True and read LAST for profiling.
_RUN_KWARGS = {}
LAST = None


def _run_spmd(nc, in_maps):
    global LAST
    from concourse.bass_utils import run_bass_kernel_spmd
    LAST = run_bass_kernel_spmd(nc, in_maps, core_ids=list(range(N_CORES)),
                                **_RUN_KWARGS)
    return LAST


def _host_patch(x):
    """Exact CPU-XLA-equivalent computation of the 9 modified columns."""
    pw = np.float32(16.0) ** np.arange(8, dtype=np.float32)
    imm = x[..., IMM0].astype(np.float32)
    pc = x[..., PC0].astype(np.float32)
    for n in range(1, 8):
        imm = (x[..., IMM0 + n] * pw[n] + imm).astype(np.float32)
        pc = (x[..., PC0 + n] * pw[n] + pc).astype(np.float32)
    axs = np.zeros(x.shape[:-1], dtype=np.int64)
    for n in range(8):
        axs += x[..., AX0 + n].astype(np.int32).astype(np.int64) * (16 ** n)
    ax = ((axs + 2**31) % 2**32 - 2**31).astype(np.int32)
    ax_is_zero = ax == 0

    any_bz = bool((x[..., OPC_BZ] > 0.5).any())

    pc8 = (pc + np.float32(8.0)).astype(np.float32)
    if any_bz:
        new_pc = np.where(ax_is_zero, imm, pc8)
        bt = ax_is_zero.astype(np.float32)
    else:  # any_bnz
        new_pc = np.where(~ax_is_zero, imm, pc8)
        bt = (~ax_is_zero).astype(np.float32)
    v = new_pc.astype(np.int32)
    shifts = np.arange(8, dtype=np.int32) * 4
    nibs = ((v[..., None] >> shifts) & 15).astype(np.float32)
    return np.concatenate([nibs, bt[..., None]], axis=-1)


def kernel(x):
    x = np.ascontiguousarray(np.asarray(x), dtype=np.float32)
    assert x.shape == (B, T, C), x.shape

    any_jmp = bool((x[..., OPC_JMP] > 0.5).any())
    any_bz = bool((x[..., OPC_BZ] > 0.5).any())
    any_bnz = bool((x[..., OPC_BNZ] > 0.5).any())
    if not (any_jmp or any_bz or any_bnz):
        return x.copy()

    xr = x.reshape(N_CORES, ROWS_PER_CORE, C)
    if any_jmp:
        nc = _get_kernel("jmp")
        nt = CONFIG["tiles"]
        wt = W // nt
        if CONFIG.get("impl") == "raw":
            # transposed layout: a[core][s, p, n, w] = row (s*P + p)*wt + w,
            # col IMM0 + n  -> every chain read on device is contiguous
            a = xr[:, :, IMM0:IMM0 + 8].reshape(N_CORES, nt, P, wt, 8)
            a = np.ascontiguousarray(a.transpose(0, 1, 2, 4, 3)).reshape(
                N_CORES, nt, P, 8 * wt)
        else:
            a = np.ascontiguousarray(xr[:, :, IMM0:IMM0 + 8])
        in_maps = [{"a": a[c]} for c in range(N_CORES)]
        res = _run_spmd(nc, in_maps)

        out = x.copy()
        orows = out.reshape(N_CORES, ROWS_PER_CORE, C)
        nt = CONFIG["tiles"]
        wt = W // nt
        for c in range(N_CORES):
            nib = res.results[c]["onib"]          # [nt, P, 8*wt] u16
            nib = nib.reshape(nt, P, 4, wt, 2)    # [t, p, plane, w, half]
            # row = (t*P + p)*wt + w ; col = 4*half + plane
            nib = nib.transpose(0, 1, 3, 4, 2).reshape(ROWS_PER_CORE, 8)
            orows[c, :, PC0:PC0 + 8] = nib.astype(np.float32)
        orows[:, :, BT] = 1.0
        return out

    nc = _get_kernel("patch")
    patch = _host_patch(x).reshape(N_CORES, ROWS_PER_CORE, 9)
    in_maps = [{"x": xr[c], "patch": patch[c]} for c in range(N_CORES)]
    res = _run_spmd(nc, in_maps)
    out = np.empty((N_CORES, ROWS_PER_CORE, C), dtype=np.float32)
    for c in range(N_CORES):
        out[c] = res.results[c]["out"]
    return out.reshape(B, T, C)
